# revision 17
# baseline (speedup 1.0000x reference)
"""GroupedQueryAttention Trainium2 kernel (8 NeuronCores).

Sharding: (batch b in 0..1) x (kv-head group g in 0..3) -> core 4*b+g.
Each core computes, for its batch, the 4 query heads (4g..4g+3) that share
kv head g, plus the partial output projection through the matching 512-row
slice of Wo.  The host sums the 4 bf16 partials per batch in f32.

On-device dataflow is fully "transposed": activations live as [feature,
token] so every matmul contraction sits on the partition axis, and the
softmax probabilities come out directly in the layout the P@V matmul
needs.  Performance structure:
  - inputs stream on all three DMA queues (scalar/sync hwdge + gpsimd
    swdge) in a schedule that delivers each operand just before its
    first consumer; outputs stream back per 128-token half-row on a
    rotating queue
  - dummy warm-up matmuls run during the initial DMA wait so the PE HAM
    clock gate is at 8/8 when real work starts
  - attention is head-PAIR merged: scores/exp/P@V process [128, 2x512]
    tiles (two query heads that share the kv head), halving tensor and
    scalar instruction counts vs per-head tiles
  - causal diagonal 512-blocks are computed on restricted query ranges
    (512/384/256/128 wide); the 0/1 triangle mask multiply touches only
    the four 128-wide blocks that actually straddle the diagonal
  - softmax denominators are accumulated on the otherwise-idle GpSimd
    (Pool) engine -- SBUF-only bf16 adds -- freeing the DVE, whose
    backlog previously stalled PSUM recycling and starved the PE;
    the single ones-matmul partition reduction covers a head pair
  - the output projection runs in [128,1024] PSUM tiles interleaved
    between the next chunk's score passes; each half-row DMAs out as
    soon as its cast completes
  - rmsnorm uses reciprocal via Ln/Exp activations off one table set
"""

import numpy as np
import ml_dtypes

DIM, H, KV, S, B = 2048, 16, 4, 2048, 2
HD = DIM // H          # 128
GQ = H // KV           # 4 query heads per kv head
P = 128                # partitions
NK = DIM // P          # 16 contraction tiles
NCH = S // 512         # 4 sequence chunks of 512
EPS = 1e-6
BF = ml_dtypes.bfloat16

_CACHED = {}


def _build_program():
    import concourse.bass as bass
    import concourse.tile as tile
    from concourse import bacc
    from concourse import mybir
    from concourse.masks import make_identity

    f32 = mybir.dt.float32
    bf16 = mybir.dt.bfloat16
    AF = mybir.ActivationFunctionType

    nc = bacc.Bacc()
    xt4 = nc.declare_dram_parameter("xt4", [P, NCH, NK, 512], bf16, isOutput=False)
    wq = nc.declare_dram_parameter("wq", [P, GQ, NK, HD], bf16, isOutput=False)
    wk = nc.declare_dram_parameter("wk", [P, NK, HD], bf16, isOutput=False)
    wv = nc.declare_dram_parameter("wv", [P, NK, HD], bf16, isOutput=False)
    wo = nc.declare_dram_parameter("wo", [P, GQ, DIM], bf16, isOutput=False)
    cosq = nc.declare_dram_parameter("cosq", [HD, S], bf16, isOutput=False)
    sinq = nc.declare_dram_parameter("sinq", [HD, S], bf16, isOutput=False)
    cosk = nc.declare_dram_parameter("cosk", [HD, S], bf16, isOutput=False)
    sink = nc.declare_dram_parameter("sink", [HD, S], bf16, isOutput=False)
    mtri = nc.declare_dram_parameter("mtri", [P, 1280], bf16, isOutput=False)
    rsw = nc.declare_dram_parameter("rsw", [P, P], bf16, isOutput=False)
    po = nc.declare_dram_parameter("po", [S, DIM], bf16, isOutput=True)

    inv_sqrt_hd = 1.0 / float(np.sqrt(HD))

    with tile.TileContext(nc) as tc:
      with tc.tile_pool(name="const", bufs=1) as const, \
           tc.tile_pool(name="w5", bufs=1) as w5, \
           tc.tile_pool(name="hatp", bufs=1) as hatp:
        ones_sb = const.tile([P, P], bf16)
        nc.vector.memset(ones_sb, 1.0)
        osb = const.tile([P, P], bf16)           # 1/HD for the rmsnorm mean
        nc.vector.memset(osb, 1.0 / HD)
        ident = const.tile([P, P], bf16)
        make_identity(nc, ident)
        wmov = const.tile([P, 512], bf16)
        nc.vector.memset(wmov, 0.0)
        tri_sb = const.tile([P, P], bf16)        # causal triangle (col >= row)
        epsb = const.tile([P, 1], f32)
        nc.vector.memset(epsb, EPS)
        # preload the natural_log_exp_and_others activation-table set (id 6):
        # it holds ln, exp, square, copy and identity, so every scalar
        # activation in the kernel runs off this one load -- no switches
        nc.scalar.add_instruction(mybir.InstLoadActFuncSet(
            name=nc.get_next_instruction_name(), act_func_set_id=6,
            ins=[], outs=[]))

        wo_sb = w5.tile([P, GQ, DIM], bf16)

        v_nat = hatp.tile([P, NK, HD], bf16, tag="vnat")
        khat = hatp.tile([P, S], bf16, tag="khat")
        qhat = hatp.tile([P, NCH, GQ, 512], bf16, tag="qhat")
        onorm = hatp.tile([P, NCH, GQ, 512], bf16, tag="onorm")
        # chunk 0's probability tiles + pair denominator accumulators live
        # at top level: its score/exp pass is hoisted into the phase-A tail
        # so phase B opens with PV work that is ready immediately
        pvs = {}
        accs = {}
        pt0 = [hatp.tile([P, 512], bf16, tag=f"pt0_{k}", name=f"pt0_{k}")
               for k in range(16)]
        acc0 = [hatp.tile([P, 1024], bf16, tag=f"acc0_{p}", name=f"acc0_{p}")
                for p in range(2)]

        # ---- warm-up: keep the PE busy while the first DMAs land ----
        with tc.tile_pool(name="wps", bufs=1, space="PSUM") as wps:
            wt = wps.tile([P, 512], f32, tag="warm")
            for _ in range(24):
                nc.tensor.matmul(wt, ones_sb, wmov, start=True, stop=True)

        # ---- phase A: projections + rmsnorm + rope, chunk-pipelined ----
        with tc.tile_pool(name="xtp", bufs=1) as xtp, \
             tc.tile_pool(name="xchk", bufs=12) as xchk, \
             tc.tile_pool(name="q32p", bufs=12) as q32p, \
             tc.tile_pool(name="vTp", bufs=2) as vTp, \
             tc.tile_pool(name="scr", bufs=3) as scr, \
             tc.tile_pool(name="psA", bufs=4, space="PSUM") as psA, \
             tc.tile_pool(name="psQ", bufs=3, space="PSUM") as psQ, \
             tc.tile_pool(name="psA2", bufs=1, space="PSUM") as psA2:
            wk_sb = xtp.tile([P, NK, HD], bf16, tag="wk")
            wq_sb = xtp.tile([P, GQ, NK, HD], bf16, tag="wq")
            wv_sb = xtp.tile([P, NK, HD], bf16, tag="wv")
            cs_sb = {}
            for nm in ("cosq", "sinq", "cosk", "sink"):
                cs_sb[nm] = xtp.tile([P, S], bf16, tag=f"cs_{nm}", name=f"cs_{nm}")

            xts = {}

            def alloc_chunk(c):
                xts[c] = [xchk.tile([P, 4, 512], bf16, tag="xt",
                                    name=f"xt{c}_{qn}") for qn in range(4)]

            def dma_q(eng, c, qn):
                eng.dma_start(out=xts[c][qn],
                              in_=xt4.ap()[:, c][:, qn * 4:(qn + 1) * 4])

            for c in range(3):
                alloc_chunk(c)
            # Queue model (measured): scalar+sync SHARE the hwdge DMA
            # engine (~100 GB/s combined); the gpsimd swdge queue is an
            # independent ~140 GB/s.  So the latency-critical bulk (x
            # chunks 0-1, then the k rope tables) rides swdge, the
            # projection weights ride scalar, and small/late operands
            # ride sync.
            # Queue model (measured): scalar+sync share the hwdge DMA
            # engine; gpsimd swdge is independent and fastest for bulk.
            # x chunks 0-1 and the k tables ride swdge; projection
            # weights ride scalar; small masks ride sync (chunks 2-3 are
            # issued inside the loop, gated on buffer reuse so they can't
            # steal HBM bandwidth from the startup-critical transfers).
            for qn in range(4):
                dma_q(nc.gpsimd, 0, qn)
            for qn in range(4):
                dma_q(nc.gpsimd, 1, qn)
            nc.gpsimd.dma_start(out=cs_sb["cosk"], in_=cosk[:, :])
            nc.gpsimd.dma_start(out=cs_sb["sink"], in_=sink[:, :])
            nc.scalar.dma_start(out=wk_sb, in_=wk.ap().rearrange("p j n -> p (j n)"))
            nc.scalar.dma_start(out=wv_sb, in_=wv.ap().rearrange("p j n -> p (j n)"))
            for hh in range(GQ):
                nc.scalar.dma_start(out=wq_sb[:, hh], in_=wq.ap()[:, hh])
            nc.scalar.dma_start(out=cs_sb["cosq"], in_=cosq[:, :])
            nc.scalar.dma_start(out=cs_sb["sinq"], in_=sinq[:, :])
            nc.sync.dma_start(out=tri_sb, in_=mtri[:, 0:P])
            dma_q(nc.sync, 2, 2)
            dma_q(nc.sync, 2, 3)

            def p1(c):
                xt_c = xts.pop(c)
                srcs = {}
                for slot in (4, 5, 0, 1, 2, 3):
                    ps = psA.tile([P, 512], f32, tag="proj")
                    for j in range(NK):
                        if slot < 4:
                            lhs = wq_sb[:, slot, j, :]
                        elif slot == 4:
                            lhs = wk_sb[:, j, :]
                        else:
                            lhs = wv_sb[:, j, :]
                        nc.tensor.matmul(ps, lhs, xt_c[j // 4][:, j % 4, :],
                                         start=(j == 0), stop=(j == NK - 1))
                    if slot == 5:
                        vT_c = vTp.tile([P, 512], bf16, tag="vT")
                        nc.scalar.copy(vT_c, ps)
                        tp = psA2.tile([P, 512], bf16, tag="vtr", bufs=1)
                        for u in range(4):
                            nc.tensor.transpose(tp[:, u * HD:(u + 1) * HD],
                                                vT_c[:, u * HD:(u + 1) * HD], ident)
                        nc.scalar.copy(v_nat[:, 4 * c:4 * c + 4, :], tp)
                    else:
                        t32 = q32p.tile([P, 512], bf16, tag="q32",
                                        name=f"q32_{c}_{slot}")
                        nc.scalar.copy(t32, ps)
                        srcs[slot] = t32
                return srcs

            def p2(c, srcs):
                # wave-major emission: each engine runs its stage of the
                # rmsnorm+rope chain back-to-back across a wave of slots,
                # paying the cross-engine latency once per wave instead of
                # once per slot (the per-slot chain is latency-bound and
                # starves the PE at the phase-A tail)
                sl = slice(c * 512, (c + 1) * 512)
                for wave in ((4, 0, 1), (2, 3)):
                    st = {}
                    for t in wave:
                        src = srcs[t]
                        # rotate_half is a partition rotation by 64; the
                        # sign lives in the host-prepared sin tables, so
                        # two SBUF->SBUF DMAs replace the permutation
                        # matmul (the PE is saturated in phase A, the
                        # sync DMA queue is idle)
                        rot = scr.tile([P, 512], bf16, tag="rot")
                        nc.sync.dma_start(out=rot[0:64, :], in_=src[64:128, :])
                        nc.sync.dma_start(out=rot[64:128, :], in_=src[0:64, :])
                        sqb = scr.tile([P, 512], bf16, tag="sqb")
                        nc.scalar.activation(sqb, src, AF.Square)
                        st[t] = [rot, sqb]
                    for t in wave:
                        ssq = psQ.tile([P, 512], f32, tag="ssq")
                        nc.tensor.matmul(ssq, osb, st[t][1], start=True, stop=True)
                        st[t].append(ssq)
                    for t in wave:
                        lnb = scr.tile([P, 512], f32, tag="lnb")
                        nc.scalar.activation(lnb, st[t][2], AF.Ln, bias=epsb)
                        st[t].append(lnb)
                    for t in wave:
                        rsb = scr.tile([P, 512], bf16, tag="rsb")
                        nc.scalar.activation(rsb, st[t][3], AF.Exp, scale=-0.5)
                        st[t].append(rsb)
                    for t in wave:
                        src = srcs[t]
                        cosT = cs_sb["cosq" if t < 4 else "cosk"]
                        t1 = scr.tile([P, 512], bf16, tag="t1")
                        nc.vector.tensor_mul(t1, src, cosT[:, sl])
                        st[t].append(t1)
                    for t in wave:
                        sinT = cs_sb["sinq" if t < 4 else "sink"]
                        t2 = scr.tile([P, 512], bf16, tag="t2")
                        nc.vector.tensor_mul(t2, st[t][0], sinT[:, sl])
                        st[t].append(t2)
                    for t in wave:
                        t3 = scr.tile([P, 512], bf16, tag="t3")
                        nc.vector.tensor_add(t3, st[t][5], st[t][6])
                        st[t].append(t3)
                    for t in wave:
                        dst = qhat[:, c, t, :] if t < 4 else khat[:, sl]
                        nc.vector.tensor_mul(dst, st[t][7], st[t][4])

            # software-pipelined: P2 for chunk c-1 is emitted after P1 for
            # chunk c, so its small matmuls never head-of-line block P1
            prev = None
            for c in range(NCH):
                if c == 0:
                    alloc_chunk(3)
                    dma_q(nc.gpsimd, 2, 0)
                    dma_q(nc.gpsimd, 2, 1)
                    dma_q(nc.gpsimd, 3, 0)
                    dma_q(nc.gpsimd, 3, 1)
                elif c == 1:
                    dma_q(nc.gpsimd, 3, 2)
                    dma_q(nc.gpsimd, 3, 3)
                    nc.scalar.dma_start(
                        out=wo_sb, in_=wo.ap().rearrange("p h n -> p (h n)"))
                cur = (c, p1(c))
                if prev is not None:
                    p2(*prev)
                prev = cur
            p2(*prev)

            # hoisted chunk-0 attention pass 1 (its 4 diagonal tiles per
            # head), reusing the projection PSUM slots; denominator pair
            # accumulators build on the DVE
            for pr in range(2):
                pv_list = []
                for hp in range(2):
                    h = 2 * pr + hp
                    for u in range(4):
                        w = 512 - 128 * u
                        sp = psA.tile([P, 512], f32, tag="proj")
                        nc.tensor.matmul(sp[:, 0:w], khat[:, u * P:(u + 1) * P],
                                         qhat[:, 0, h, 128 * u:512],
                                         start=True, stop=True)
                        pt = pt0[4 * h + u]
                        nc.scalar.activation(pt[:, 0:w], sp[:, 0:w], AF.Exp,
                                             scale=inv_sqrt_hd)
                        nc.vector.tensor_mul(pt[:, 0:128], pt[:, 0:128], tri_sb)
                        oo = hp * 512 + 128 * u
                        if u == 0:
                            nc.vector.tensor_copy(
                                acc0[pr][:, oo:oo + 512], pt[:, 0:512])
                        else:
                            nc.vector.tensor_add(
                                acc0[pr][:, oo:oo + w],
                                acc0[pr][:, oo:oo + w], pt[:, 0:w])
                        pv_list.append((u, pt, 0, oo, w, u == 0, u == 3))
                pvs[(0, pr)] = pv_list
                accs[(0, pr)] = acc0[pr]

        # ---- phase B: attention + output projection, per chunk ----
        with tc.tile_pool(name="ptp", bufs=34) as ptp, \
             tc.tile_pool(name="accp", bufs=3) as accp, \
             tc.tile_pool(name="recp", bufs=2) as recp, \
             tc.tile_pool(name="rowp", bufs=4) as rowp, \
             tc.tile_pool(name="psc", bufs=2, space="PSUM") as psc, \
             tc.tile_pool(name="pvd", bufs=2, space="PSUM") as pvd:
            qrot = [0]

            def pass1(c, pr):
                # pv entries: (key_tile, pt, col_off, out_off, width, start, stop)
                sl = slice(c * 512, (c + 1) * 512)
                pv_list = []
                for j in range(4 * c):
                    sc = psc.tile([P, 1024], f32, tag="sc",
                                  name=f"sc_{c}_{pr}_{j}")
                    for hp in range(2):
                        nc.tensor.matmul(sc[:, hp * 512:(hp + 1) * 512],
                                         khat[:, j * P:(j + 1) * P],
                                         qhat[:, c, 2 * pr + hp, :],
                                         start=True, stop=True)
                    pt = ptp.tile([P, 1024], bf16, tag="pt",
                                  name=f"pt_{c}_{pr}_{j}")
                    nc.scalar.activation(pt, sc, AF.Exp, scale=inv_sqrt_hd)
                    for hp in range(2):
                        pv_list.append((j, pt, hp * 512, hp * 512, 512,
                                        j == 0, False))
                # diagonal 512-block per head: restricted query ranges;
                # tile u covers queries [128u, 512) of the chunk
                for hp in range(2):
                    h = 2 * pr + hp
                    qh = qhat[:, c, h, :]
                    scA = psc.tile([P, 1024], f32, tag="sc",
                                   name=f"scA_{c}_{h}")
                    nc.tensor.matmul(scA[:, 0:512],
                                     khat[:, (4 * c) * P:(4 * c + 1) * P],
                                     qh[:, 0:512],
                                     start=True, stop=True)
                    nc.tensor.matmul(scA[:, 512:896],
                                     khat[:, (4 * c + 1) * P:(4 * c + 2) * P],
                                     qh[:, 128:512],
                                     start=True, stop=True)
                    ptA = ptp.tile([P, 1024], bf16, tag="pt",
                                   name=f"ptA_{c}_{h}")
                    nc.scalar.activation(ptA[:, 0:896], scA[:, 0:896],
                                         AF.Exp, scale=inv_sqrt_hd)
                    nc.vector.tensor_mul(ptA[:, 0:128], ptA[:, 0:128], tri_sb)
                    nc.vector.tensor_mul(ptA[:, 512:640], ptA[:, 512:640], tri_sb)
                    scB = psc.tile([P, 1024], f32, tag="sc",
                                   name=f"scB_{c}_{h}")
                    nc.tensor.matmul(scB[:, 0:256],
                                     khat[:, (4 * c + 2) * P:(4 * c + 3) * P],
                                     qh[:, 256:512],
                                     start=True, stop=True)
                    nc.tensor.matmul(scB[:, 256:384],
                                     khat[:, (4 * c + 3) * P:(4 * c + 4) * P],
                                     qh[:, 384:512],
                                     start=True, stop=True)
                    ptB = ptp.tile([P, 1024], bf16, tag="pt",
                                   name=f"ptB_{c}_{h}")
                    nc.scalar.activation(ptB[:, 0:384], scB[:, 0:384],
                                         AF.Exp, scale=inv_sqrt_hd)
                    nc.vector.tensor_mul(ptB[:, 0:128], ptB[:, 0:128], tri_sb)
                    nc.vector.tensor_mul(ptB[:, 256:384], ptB[:, 256:384], tri_sb)
                    oo = hp * 512
                    pv_list += [(4 * c + 0, ptA, 0, oo + 0, 512, c == 0, False),
                                (4 * c + 1, ptA, 512, oo + 128, 384, False, False),
                                (4 * c + 2, ptB, 0, oo + 256, 256, False, False),
                                (4 * c + 3, ptB, 256, oo + 384, 128, False, False)]
                # close the accumulation group of each PSUM half (bank)
                for hp in range(2):
                    for idx in range(len(pv_list) - 1, -1, -1):
                        if pv_list[idx][3] // 512 == hp:
                            pv_list[idx] = pv_list[idx][:6] + (True,)
                            break
                pvs[(c, pr)] = pv_list
                # denominator: accumulate probability tiles on the Pool
                # engine (SBUF-only bf16 adds; GpSimd has no PSUM port)
                acc = accp.tile([P, 1024], bf16, tag="acc", name=f"acc_{c}_{pr}")
                init = [False, False]
                for idx, (j, pt, co, oo, w, _, _) in enumerate(pv_list):
                    if j < 4 * c:
                        if co != 0:
                            continue  # half 1, covered by the wide op below
                        if not init[0]:
                            nc.vector.tensor_copy(acc, pt)
                            init = [True, True]
                        else:
                            # off-diag pair tile: one wide add for both heads
                            nc.vector.tensor_add(acc, acc, pt)
                    elif not init[oo // 512]:
                        nc.vector.tensor_copy(acc[:, oo:oo + w], pt[:, co:co + w])
                        init[oo // 512] = True
                    else:
                        nc.vector.tensor_add(acc[:, oo:oo + w],
                                             acc[:, oo:oo + w], pt[:, co:co + w])
                accs[(c, pr)] = acc

            def pass2(c, pr):
                # partition-reduce den for the head pair, P@V, normalize
                sl = slice(c * 512, (c + 1) * 512)
                pv_list = pvs.pop((c, pr))
                acc = accs.pop((c, pr))
                den = pvd.tile([P, 1024], f32, tag="pvd", name=f"den_{c}_{pr}")
                for hp in range(2):
                    nc.tensor.matmul(den[:, hp * 512:(hp + 1) * 512], ones_sb,
                                     acc[:, hp * 512:(hp + 1) * 512],
                                     start=True, stop=True)
                rec = recp.tile([P, 1024], f32, tag="rec")
                nc.vector.reciprocal_approx_fast(out=rec, in_=den)

                ots = pvd.tile([P, 1024], f32, tag="pvd", name=f"ot_{c}_{pr}")
                for (j, pt, co, oo, w, st, sp) in pv_list:
                    nc.tensor.matmul(ots[:, oo:oo + w], v_nat[:, j, :],
                                     pt[:, co:co + w], start=st, stop=sp)
                nc.vector.tensor_mul(onorm[:, c, 2 * pr:2 * pr + 2, :], ots, rec)

            def emit_po(c, half):
                # output projection for two of chunk c's token tiles; each
                # 1024-wide half-row leaves on a rotating DMA queue as soon
                # as its cast lands
                for i in (4 * c + 2 * half, 4 * c + 2 * half + 1):
                    isl = slice(i * P, (i + 1) * P)
                    for n2 in range(2):
                        if c == NCH - 1 and (2 * i + n2) % 2 == 1:
                            ps = psc.tile([P, 1024], f32, tag="sc",
                                          name=f"po_{i}_{n2}")
                        else:
                            ps = pvd.tile([P, 1024], f32, tag="pvd",
                                          name=f"po_{i}_{n2}")
                        for q5 in range(2):
                            n5 = n2 * 1024 + q5 * 512
                            for h in range(GQ):
                                nc.tensor.matmul(
                                    ps[:, q5 * 512:(q5 + 1) * 512],
                                    onorm[:, i // 4, h, (i % 4) * P:(i % 4 + 1) * P],
                                    wo_sb[:, h, n5:n5 + 512],
                                    start=(h == 0), stop=(h == GQ - 1))
                        row = rowp.tile([P, 1024], bf16, tag="row",
                                        name=f"row_{i}_{n2}")
                        if (2 * i + n2) % 2 == 0:
                            nc.vector.tensor_copy(row, ps)
                        else:
                            nc.scalar.copy(row, ps)
                        eng = (nc.gpsimd, nc.scalar, nc.sync)[qrot[0] % 3]
                        qrot[0] += 1
                        eng.dma_start(out=po[isl, n2 * 1024:(n2 + 1) * 1024],
                                      in_=row)

            for c in range(NCH):
                if c == 0:
                    pass2(0, 0)
                    pass2(0, 1)
                else:
                    pass1(c, 0)
                    emit_po(c - 1, 0)
                    pass1(c, 1)
                    pass2(c, 0)
                    emit_po(c - 1, 1)
                    pass2(c, 1)
            emit_po(NCH - 1, 0)
            emit_po(NCH - 1, 1)
    nc.compile()
    return nc


def _causal_ok(mask):
    m = np.asarray(mask).reshape(S, S)
    tri = np.tril(np.ones((S, S), dtype=bool))
    return bool(np.all(m[tri] == 0.0) and np.all(m[~tri] <= -1e8))


def _reference_fallback(x, Wq, Wk, Wv, Wo, qg, kg, cos, sin, mask):
    x64 = np.asarray(x, dtype=np.float32)
    q = (x64 @ Wq).reshape(B, S, H, HD).transpose(0, 2, 1, 3)
    k = (x64 @ Wk).reshape(B, S, KV, HD).transpose(0, 2, 1, 3)
    v = (x64 @ Wv).reshape(B, S, KV, HD).transpose(0, 2, 1, 3)

    def rms(t, g):
        r = np.sqrt(np.mean(t * t, axis=-1, keepdims=True) + EPS)
        return g * (t / r)

    q, k = rms(q, qg), rms(k, kg)

    def rot(t):
        return np.concatenate([-t[..., HD // 2:], t[..., :HD // 2]], axis=-1)

    c = cos[None, None, :, :]
    s = sin[None, None, :, :]
    q = q * c + rot(q) * s
    k = k * c + rot(k) * s
    k = np.repeat(k, GQ, axis=1)
    v = np.repeat(v, GQ, axis=1)
    sc = np.einsum('bhqd,bhkd->bhqk', q, k) / np.sqrt(HD) + np.asarray(mask).reshape(1, 1, S, S)
    sc = sc - sc.max(axis=-1, keepdims=True)
    e = np.exp(sc)
    a = e / e.sum(axis=-1, keepdims=True)
    o = np.einsum('bhqk,bhkd->bhqd', a, v)
    o = o.transpose(0, 2, 1, 3).reshape(B, S, H * HD)
    return (o @ Wo).astype(np.float32)


def kernel(x, Wq, Wk, Wv, Wo, qg, kg, cos, sin, mask, **_unused):
    x = np.asarray(x, dtype=np.float32)
    Wq, Wk, Wv, Wo = (np.asarray(a, dtype=np.float32) for a in (Wq, Wk, Wv, Wo))
    qg, kg = np.asarray(qg, np.float32), np.asarray(kg, np.float32)
    cos, sin = np.asarray(cos, np.float32), np.asarray(sin, np.float32)
    if not _causal_ok(mask):
        return _reference_fallback(x, Wq, Wk, Wv, Wo, qg, kg, cos, sin, mask)

    from concourse.bass_utils import run_bass_kernel_spmd

    if "nc" not in _CACHED:
        _CACHED["nc"] = _build_program()
    nc = _CACHED["nc"]

    cosT = np.ascontiguousarray(cos.T)  # [HD, S]
    sinT = np.ascontiguousarray(sin.T)

    # rope via halves: out[:64] = x[:64]*cos[:64] + x[64:]*sin_tbl[:64]
    #                  out[64:] = x[64:]*cos[64:] + x[:64]*sin_tbl[64:]
    # reference: rot(x)[:64] = -x[64:], rot(x)[64:] = x[:64]; gains fold in.
    def tables(g):
        ct = cosT * g[:, None]
        st = np.empty_like(sinT)
        st[:64] = -sinT[:64] * g[64:, None]
        st[64:] = sinT[64:] * g[:64, None]
        return ct.astype(BF), st.astype(BF)

    cq, sq = tables(qg)
    ck, sk = tables(kg)

    rsw = np.zeros((P, P), dtype=np.float32)
    for i in range(P):
        rsw[i, (i + 64) % P] = 1.0
    rsw = rsw.astype(BF)

    # restricted-diagonal masks: within each 128-column sub-range that
    # starts a diagonal tile, query-col >= key-row; elsewhere 1.
    rows = np.arange(P)[:, None]
    tri = (np.arange(P)[None, :] >= rows)          # [128,128] step
    onesP = np.ones((P, P), dtype=bool)
    mA = np.concatenate([tri, onesP, onesP, onesP, tri, onesP, onesP], axis=1)  # 896
    mB = np.concatenate([tri, onesP, tri], axis=1)                              # 384
    mtri = np.concatenate([mA, mB], axis=1).astype(BF)                          # [128,1280]

    def part_layout(w, cols):
        # [DIM, cols] -> [P, NK, cols] with feature d = j*128 + p
        return np.ascontiguousarray(w.reshape(NK, P, cols).transpose(1, 0, 2)).astype(BF)

    xt4 = []
    for b in range(B):
        xT = x[b].T  # [DIM, S]
        xt4.append(np.ascontiguousarray(
            xT.reshape(NK, P, NCH, 512).transpose(1, 2, 0, 3)).astype(BF))

    in_maps = []
    for core in range(8):
        b, g = divmod(core, KV)
        wo_g = Wo[g * GQ * HD:(g + 1) * GQ * HD, :]
        wq_g = Wq[:, g * GQ * HD:(g + 1) * GQ * HD]  # [DIM, 4*HD]
        in_maps.append({
            "xt4": xt4[b],
            # head-major [P, GQ, NK, HD] so each head's slice is one
            # contiguous per-partition DMA
            "wq": np.ascontiguousarray(
                wq_g.reshape(NK, P, GQ, HD).transpose(1, 2, 0, 3)).astype(BF),
            "wk": part_layout(Wk[:, g * HD:(g + 1) * HD], HD),
            "wv": part_layout(Wv[:, g * HD:(g + 1) * HD], HD),
            "wo": np.ascontiguousarray(
                wo_g.reshape(GQ, P, DIM).transpose(1, 0, 2)).astype(BF),
            "cosq": cq, "sinq": sq, "cosk": ck, "sink": sk,
            "mtri": mtri, "rsw": rsw,
        })

    res = run_bass_kernel_spmd(nc, in_maps, list(range(8)))
    out = np.zeros((B, S, DIM), dtype=np.float32)
    for core in range(8):
        out[core // KV] += res.results[core]["po"].astype(np.float32)
    return out


# revision 19
# speedup vs baseline: 1.0014x; 1.0014x over previous
"""GroupedQueryAttention Trainium2 kernel (8 NeuronCores).

Sharding: (batch b in 0..1) x (kv-head group g in 0..3) -> core 4*b+g.
Each core computes, for its batch, the 4 query heads (4g..4g+3) that share
kv head g, plus the partial output projection through the matching 512-row
slice of Wo.  The host sums the 4 bf16 partials per batch in f32.

On-device dataflow is fully "transposed": activations live as [feature,
token] so every matmul contraction sits on the partition axis, and the
softmax probabilities come out directly in the layout the P@V matmul
needs.  Performance structure:
  - inputs stream on all three DMA queues (scalar/sync hwdge + gpsimd
    swdge) in a schedule that delivers each operand just before its
    first consumer; outputs stream back per 128-token half-row on a
    rotating queue
  - dummy warm-up matmuls run during the initial DMA wait so the PE HAM
    clock gate is at 8/8 when real work starts
  - attention is head-PAIR merged: scores/exp/P@V process [128, 2x512]
    tiles (two query heads that share the kv head), halving tensor and
    scalar instruction counts vs per-head tiles
  - causal diagonal 512-blocks are computed on restricted query ranges
    (512/384/256/128 wide); the 0/1 triangle mask multiply touches only
    the four 128-wide blocks that actually straddle the diagonal
  - softmax denominators are accumulated on the otherwise-idle GpSimd
    (Pool) engine -- SBUF-only bf16 adds -- freeing the DVE, whose
    backlog previously stalled PSUM recycling and starved the PE;
    the single ones-matmul partition reduction covers a head pair
  - the output projection runs in [128,1024] PSUM tiles interleaved
    between the next chunk's score passes; each half-row DMAs out as
    soon as its cast completes
  - rmsnorm uses reciprocal via Ln/Exp activations off one table set
"""

import numpy as np
import ml_dtypes

DIM, H, KV, S, B = 2048, 16, 4, 2048, 2
HD = DIM // H          # 128
GQ = H // KV           # 4 query heads per kv head
P = 128                # partitions
NK = DIM // P          # 16 contraction tiles
NCH = S // 512         # 4 sequence chunks of 512
EPS = 1e-6
BF = ml_dtypes.bfloat16

_CACHED = {}


def _build_program():
    import concourse.bass as bass
    import concourse.tile as tile
    from concourse import bacc
    from concourse import mybir
    from concourse.masks import make_identity

    f32 = mybir.dt.float32
    bf16 = mybir.dt.bfloat16
    AF = mybir.ActivationFunctionType

    nc = bacc.Bacc()
    xt4 = nc.declare_dram_parameter("xt4", [P, NCH, NK, 512], bf16, isOutput=False)
    wq = nc.declare_dram_parameter("wq", [P, GQ, NK, HD], bf16, isOutput=False)
    wk = nc.declare_dram_parameter("wk", [P, NK, HD], bf16, isOutput=False)
    wv = nc.declare_dram_parameter("wv", [P, NK, HD], bf16, isOutput=False)
    wo = nc.declare_dram_parameter("wo", [P, GQ, DIM], bf16, isOutput=False)
    cosq = nc.declare_dram_parameter("cosq", [HD, S], bf16, isOutput=False)
    sinq = nc.declare_dram_parameter("sinq", [HD, S], bf16, isOutput=False)
    cosk = nc.declare_dram_parameter("cosk", [HD, S], bf16, isOutput=False)
    sink = nc.declare_dram_parameter("sink", [HD, S], bf16, isOutput=False)
    mtri = nc.declare_dram_parameter("mtri", [P, 1280], bf16, isOutput=False)
    rsw = nc.declare_dram_parameter("rsw", [P, P], bf16, isOutput=False)
    po = nc.declare_dram_parameter("po", [S, DIM], bf16, isOutput=True)

    inv_sqrt_hd = 1.0 / float(np.sqrt(HD))

    with tile.TileContext(nc) as tc:
      with tc.tile_pool(name="const", bufs=1) as const, \
           tc.tile_pool(name="w5", bufs=1) as w5, \
           tc.tile_pool(name="hatp", bufs=1) as hatp:
        ones_sb = const.tile([P, P], bf16)
        nc.vector.memset(ones_sb, 1.0)
        osb = const.tile([P, P], bf16)           # 1/HD for the rmsnorm mean
        nc.vector.memset(osb, 1.0 / HD)
        ident = const.tile([P, P], bf16)
        make_identity(nc, ident)
        wmov = const.tile([P, 512], bf16)
        nc.vector.memset(wmov, 0.0)
        tri_sb = const.tile([P, P], bf16)        # causal triangle (col >= row)
        epsb = const.tile([P, 1], f32)
        nc.vector.memset(epsb, EPS)
        # preload the natural_log_exp_and_others activation-table set (id 6):
        # it holds ln, exp, square, copy and identity, so every scalar
        # activation in the kernel runs off this one load -- no switches
        nc.scalar.add_instruction(mybir.InstLoadActFuncSet(
            name=nc.get_next_instruction_name(), act_func_set_id=6,
            ins=[], outs=[]))

        wo_sb = w5.tile([P, GQ, DIM], bf16)

        v_nat = hatp.tile([P, NK, HD], bf16, tag="vnat")
        khat = hatp.tile([P, S], bf16, tag="khat")
        qhat = hatp.tile([P, NCH, GQ, 512], bf16, tag="qhat")
        onorm = hatp.tile([P, NCH, GQ, 512], bf16, tag="onorm")
        # chunk 0's probability tiles + pair denominator accumulators live
        # at top level: its score/exp pass is hoisted into the phase-A tail
        # so phase B opens with PV work that is ready immediately
        pvs = {}
        accs = {}
        pt0 = [hatp.tile([P, 512], bf16, tag=f"pt0_{k}", name=f"pt0_{k}")
               for k in range(16)]
        acc0 = [hatp.tile([P, 1024], bf16, tag=f"acc0_{p}", name=f"acc0_{p}")
                for p in range(2)]

        # ---- warm-up: keep the PE busy while the first DMAs land ----
        with tc.tile_pool(name="wps", bufs=1, space="PSUM") as wps:
            wt = wps.tile([P, 512], f32, tag="warm")
            for _ in range(12):
                nc.tensor.matmul(wt, ones_sb, wmov, start=True, stop=True)
            # fine-grained tail: keeps the HAM clock gate at 8/8 through
            # the x-chunk DMA wait while adding at most ~one 128-column
            # matmul of latency once real data lands
            for _ in range(48):
                nc.tensor.matmul(wt[:, 0:128], ones_sb, wmov[:, 0:128],
                                 start=True, stop=True)

        # ---- phase A: projections + rmsnorm + rope, chunk-pipelined ----
        with tc.tile_pool(name="xtp", bufs=1) as xtp, \
             tc.tile_pool(name="xchk", bufs=12) as xchk, \
             tc.tile_pool(name="q32p", bufs=12) as q32p, \
             tc.tile_pool(name="vTp", bufs=2) as vTp, \
             tc.tile_pool(name="scr", bufs=3) as scr, \
             tc.tile_pool(name="psA", bufs=3, space="PSUM") as psA, \
             tc.tile_pool(name="psQ", bufs=3, space="PSUM") as psQ, \
             tc.tile_pool(name="psA2", bufs=1, space="PSUM") as psA2:
            wk_sb = xtp.tile([P, NK, HD], bf16, tag="wk")
            wq_sb = xtp.tile([P, GQ, NK, HD], bf16, tag="wq")
            wv_sb = xtp.tile([P, NK, HD], bf16, tag="wv")
            cs_sb = {}
            for nm in ("cosq", "sinq", "cosk", "sink"):
                cs_sb[nm] = xtp.tile([P, S], bf16, tag=f"cs_{nm}", name=f"cs_{nm}")

            xts = {}

            def alloc_chunk(c):
                xts[c] = [xchk.tile([P, 4, 512], bf16, tag="xt",
                                    name=f"xt{c}_{qn}") for qn in range(4)]

            def dma_q(eng, c, qn):
                eng.dma_start(out=xts[c][qn],
                              in_=xt4.ap()[:, c][:, qn * 4:(qn + 1) * 4])

            for c in range(3):
                alloc_chunk(c)
            # Queue model (measured): scalar+sync SHARE the hwdge DMA
            # engine (~100 GB/s combined); the gpsimd swdge queue is an
            # independent ~140 GB/s.  So the latency-critical bulk (x
            # chunks 0-1, then the k rope tables) rides swdge, the
            # projection weights ride scalar, and small/late operands
            # ride sync.
            # Queue model (measured): scalar+sync share the hwdge DMA
            # engine; gpsimd swdge is independent and fastest for bulk.
            # x chunks 0-1 and the k tables ride swdge; projection
            # weights ride scalar; small masks ride sync (chunks 2-3 are
            # issued inside the loop, gated on buffer reuse so they can't
            # steal HBM bandwidth from the startup-critical transfers).
            for qn in range(4):
                dma_q(nc.gpsimd, 0, qn)
            for qn in range(4):
                dma_q(nc.gpsimd, 1, qn)
            nc.gpsimd.dma_start(out=cs_sb["cosk"], in_=cosk[:, :])
            nc.gpsimd.dma_start(out=cs_sb["sink"], in_=sink[:, :])
            nc.scalar.dma_start(out=wk_sb, in_=wk.ap().rearrange("p j n -> p (j n)"))
            nc.scalar.dma_start(out=wv_sb, in_=wv.ap().rearrange("p j n -> p (j n)"))
            for hh in range(GQ):
                nc.scalar.dma_start(out=wq_sb[:, hh], in_=wq.ap()[:, hh])
            nc.scalar.dma_start(out=cs_sb["cosq"], in_=cosq[:, :])
            nc.scalar.dma_start(out=cs_sb["sinq"], in_=sinq[:, :])
            nc.sync.dma_start(out=tri_sb, in_=mtri[:, 0:P])
            dma_q(nc.sync, 2, 2)
            dma_q(nc.sync, 2, 3)

            def p1(c):
                xt_c = xts.pop(c)
                srcs = {}
                for slot in (4, 5, 0, 1, 2, 3):
                    ps = psA.tile([P, 512], f32, tag="proj")
                    for j in range(NK):
                        if slot < 4:
                            lhs = wq_sb[:, slot, j, :]
                        elif slot == 4:
                            lhs = wk_sb[:, j, :]
                        else:
                            lhs = wv_sb[:, j, :]
                        nc.tensor.matmul(ps, lhs, xt_c[j // 4][:, j % 4, :],
                                         start=(j == 0), stop=(j == NK - 1))
                    if slot == 5:
                        vT_c = vTp.tile([P, 512], bf16, tag="vT")
                        nc.scalar.copy(vT_c, ps)
                        tp = psA2.tile([P, 512], bf16, tag="vtr", bufs=1)
                        for u in range(4):
                            nc.tensor.transpose(tp[:, u * HD:(u + 1) * HD],
                                                vT_c[:, u * HD:(u + 1) * HD], ident)
                        nc.scalar.copy(v_nat[:, 4 * c:4 * c + 4, :], tp)
                    else:
                        t32 = q32p.tile([P, 512], bf16, tag="q32",
                                        name=f"q32_{c}_{slot}")
                        nc.scalar.copy(t32, ps)
                        srcs[slot] = t32
                return srcs

            def p2(c, srcs):
                # wave-major emission: each engine runs its stage of the
                # rmsnorm+rope chain back-to-back across a wave of slots,
                # paying the cross-engine latency once per wave instead of
                # once per slot (the per-slot chain is latency-bound and
                # starves the PE at the phase-A tail)
                sl = slice(c * 512, (c + 1) * 512)
                for wave in ((4, 0, 1), (2, 3)):
                    st = {}
                    for t in wave:
                        src = srcs[t]
                        # rotate_half is a partition rotation by 64; the
                        # sign lives in the host-prepared sin tables, so
                        # two SBUF->SBUF DMAs replace the permutation
                        # matmul (the PE is saturated in phase A, the
                        # sync DMA queue is idle)
                        rot = scr.tile([P, 512], bf16, tag="rot")
                        nc.sync.dma_start(out=rot[0:64, :], in_=src[64:128, :])
                        nc.sync.dma_start(out=rot[64:128, :], in_=src[0:64, :])
                        sqb = scr.tile([P, 512], bf16, tag="sqb")
                        nc.scalar.activation(sqb, src, AF.Square)
                        st[t] = [rot, sqb]
                    for t in wave:
                        ssq = psQ.tile([P, 512], f32, tag="ssq")
                        nc.tensor.matmul(ssq, osb, st[t][1], start=True, stop=True)
                        st[t].append(ssq)
                    for t in wave:
                        lnb = scr.tile([P, 512], f32, tag="lnb")
                        nc.scalar.activation(lnb, st[t][2], AF.Ln, bias=epsb)
                        st[t].append(lnb)
                    for t in wave:
                        rsb = scr.tile([P, 512], bf16, tag="rsb")
                        nc.scalar.activation(rsb, st[t][3], AF.Exp, scale=-0.5)
                        st[t].append(rsb)
                    for t in wave:
                        src = srcs[t]
                        cosT = cs_sb["cosq" if t < 4 else "cosk"]
                        t1 = scr.tile([P, 512], bf16, tag="t1")
                        nc.vector.tensor_mul(t1, src, cosT[:, sl])
                        st[t].append(t1)
                    for t in wave:
                        sinT = cs_sb["sinq" if t < 4 else "sink"]
                        t2 = scr.tile([P, 512], bf16, tag="t2")
                        nc.vector.tensor_mul(t2, st[t][0], sinT[:, sl])
                        st[t].append(t2)
                    for t in wave:
                        t3 = scr.tile([P, 512], bf16, tag="t3")
                        nc.vector.tensor_add(t3, st[t][5], st[t][6])
                        st[t].append(t3)
                    for t in wave:
                        dst = qhat[:, c, t, :] if t < 4 else khat[:, sl]
                        nc.vector.tensor_mul(dst, st[t][7], st[t][4])

            # software-pipelined: P2 for chunk c-1 is emitted after P1 for
            # chunk c, so its small matmuls never head-of-line block P1
            prev = None
            for c in range(NCH):
                if c == 0:
                    alloc_chunk(3)
                    dma_q(nc.gpsimd, 2, 0)
                    dma_q(nc.gpsimd, 2, 1)
                    dma_q(nc.gpsimd, 3, 0)
                    dma_q(nc.gpsimd, 3, 1)
                elif c == 1:
                    dma_q(nc.gpsimd, 3, 2)
                    dma_q(nc.gpsimd, 3, 3)
                    nc.scalar.dma_start(
                        out=wo_sb, in_=wo.ap().rearrange("p h n -> p (h n)"))
                cur = (c, p1(c))
                if prev is not None:
                    p2(*prev)
                prev = cur
            p2(*prev)

            # hoisted chunk-0 attention pass 1 (its 4 diagonal tiles per
            # head), reusing the projection PSUM slots; denominator pair
            # accumulators build on the DVE
            for pr in range(2):
                pv_list = []
                for hp in range(2):
                    h = 2 * pr + hp
                    for u in range(4):
                        w = 512 - 128 * u
                        sp = psA.tile([P, 512], f32, tag="proj")
                        nc.tensor.matmul(sp[:, 0:w], khat[:, u * P:(u + 1) * P],
                                         qhat[:, 0, h, 128 * u:512],
                                         start=True, stop=True)
                        pt = pt0[4 * h + u]
                        nc.scalar.activation(pt[:, 0:w], sp[:, 0:w], AF.Exp,
                                             scale=inv_sqrt_hd)
                        nc.vector.tensor_mul(pt[:, 0:128], pt[:, 0:128], tri_sb)
                        oo = hp * 512 + 128 * u
                        if u == 0:
                            nc.vector.tensor_copy(
                                acc0[pr][:, oo:oo + 512], pt[:, 0:512])
                        else:
                            nc.vector.tensor_add(
                                acc0[pr][:, oo:oo + w],
                                acc0[pr][:, oo:oo + w], pt[:, 0:w])
                        pv_list.append((u, pt, 0, oo, w, u == 0, u == 3))
                pvs[(0, pr)] = pv_list
                accs[(0, pr)] = acc0[pr]

        # ---- phase B: attention + output projection, per chunk ----
        with tc.tile_pool(name="ptp", bufs=34) as ptp, \
             tc.tile_pool(name="accp", bufs=3) as accp, \
             tc.tile_pool(name="recp", bufs=2) as recp, \
             tc.tile_pool(name="rowp", bufs=4) as rowp, \
             tc.tile_pool(name="psc", bufs=2, space="PSUM") as psc, \
             tc.tile_pool(name="pvd", bufs=2, space="PSUM") as pvd:
            qrot = [0]

            def pass1(c, pr):
                # pv entries: (key_tile, pt, col_off, out_off, width, start, stop)
                sl = slice(c * 512, (c + 1) * 512)
                pv_list = []
                for j in range(4 * c):
                    sc = psc.tile([P, 1024], f32, tag="sc",
                                  name=f"sc_{c}_{pr}_{j}")
                    for hp in range(2):
                        nc.tensor.matmul(sc[:, hp * 512:(hp + 1) * 512],
                                         khat[:, j * P:(j + 1) * P],
                                         qhat[:, c, 2 * pr + hp, :],
                                         start=True, stop=True)
                    pt = ptp.tile([P, 1024], bf16, tag="pt",
                                  name=f"pt_{c}_{pr}_{j}")
                    nc.scalar.activation(pt, sc, AF.Exp, scale=inv_sqrt_hd)
                    for hp in range(2):
                        pv_list.append((j, pt, hp * 512, hp * 512, 512,
                                        j == 0, False))
                # diagonal 512-block per head: restricted query ranges;
                # tile u covers queries [128u, 512) of the chunk
                for hp in range(2):
                    h = 2 * pr + hp
                    qh = qhat[:, c, h, :]
                    scA = psc.tile([P, 1024], f32, tag="sc",
                                   name=f"scA_{c}_{h}")
                    nc.tensor.matmul(scA[:, 0:512],
                                     khat[:, (4 * c) * P:(4 * c + 1) * P],
                                     qh[:, 0:512],
                                     start=True, stop=True)
                    nc.tensor.matmul(scA[:, 512:896],
                                     khat[:, (4 * c + 1) * P:(4 * c + 2) * P],
                                     qh[:, 128:512],
                                     start=True, stop=True)
                    ptA = ptp.tile([P, 1024], bf16, tag="pt",
                                   name=f"ptA_{c}_{h}")
                    nc.scalar.activation(ptA[:, 0:896], scA[:, 0:896],
                                         AF.Exp, scale=inv_sqrt_hd)
                    nc.vector.tensor_mul(ptA[:, 0:128], ptA[:, 0:128], tri_sb)
                    nc.vector.tensor_mul(ptA[:, 512:640], ptA[:, 512:640], tri_sb)
                    scB = psc.tile([P, 1024], f32, tag="sc",
                                   name=f"scB_{c}_{h}")
                    nc.tensor.matmul(scB[:, 0:256],
                                     khat[:, (4 * c + 2) * P:(4 * c + 3) * P],
                                     qh[:, 256:512],
                                     start=True, stop=True)
                    nc.tensor.matmul(scB[:, 256:384],
                                     khat[:, (4 * c + 3) * P:(4 * c + 4) * P],
                                     qh[:, 384:512],
                                     start=True, stop=True)
                    ptB = ptp.tile([P, 1024], bf16, tag="pt",
                                   name=f"ptB_{c}_{h}")
                    nc.scalar.activation(ptB[:, 0:384], scB[:, 0:384],
                                         AF.Exp, scale=inv_sqrt_hd)
                    nc.vector.tensor_mul(ptB[:, 0:128], ptB[:, 0:128], tri_sb)
                    nc.vector.tensor_mul(ptB[:, 256:384], ptB[:, 256:384], tri_sb)
                    oo = hp * 512
                    pv_list += [(4 * c + 0, ptA, 0, oo + 0, 512, c == 0, False),
                                (4 * c + 1, ptA, 512, oo + 128, 384, False, False),
                                (4 * c + 2, ptB, 0, oo + 256, 256, False, False),
                                (4 * c + 3, ptB, 256, oo + 384, 128, False, False)]
                # close the accumulation group of each PSUM half (bank)
                for hp in range(2):
                    for idx in range(len(pv_list) - 1, -1, -1):
                        if pv_list[idx][3] // 512 == hp:
                            pv_list[idx] = pv_list[idx][:6] + (True,)
                            break
                pvs[(c, pr)] = pv_list
                # denominator: accumulate probability tiles on the Pool
                # engine (SBUF-only bf16 adds; GpSimd has no PSUM port)
                acc = accp.tile([P, 1024], bf16, tag="acc", name=f"acc_{c}_{pr}")
                init = [False, False]
                for idx, (j, pt, co, oo, w, _, _) in enumerate(pv_list):
                    if j < 4 * c:
                        if co != 0:
                            continue  # half 1, covered by the wide op below
                        if not init[0]:
                            nc.vector.tensor_copy(acc, pt)
                            init = [True, True]
                        else:
                            # off-diag pair tile: one wide add for both heads
                            nc.vector.tensor_add(acc, acc, pt)
                    elif not init[oo // 512]:
                        nc.vector.tensor_copy(acc[:, oo:oo + w], pt[:, co:co + w])
                        init[oo // 512] = True
                    else:
                        nc.vector.tensor_add(acc[:, oo:oo + w],
                                             acc[:, oo:oo + w], pt[:, co:co + w])
                accs[(c, pr)] = acc

            def pass2(c, pr):
                # partition-reduce den for the head pair, P@V, normalize
                sl = slice(c * 512, (c + 1) * 512)
                pv_list = pvs.pop((c, pr))
                acc = accs.pop((c, pr))
                den = pvd.tile([P, 1024], f32, tag="pvd", name=f"den_{c}_{pr}")
                for hp in range(2):
                    nc.tensor.matmul(den[:, hp * 512:(hp + 1) * 512], ones_sb,
                                     acc[:, hp * 512:(hp + 1) * 512],
                                     start=True, stop=True)
                rec = recp.tile([P, 1024], f32, tag="rec")
                nc.vector.reciprocal_approx_fast(out=rec, in_=den)

                ots = pvd.tile([P, 1024], f32, tag="pvd", name=f"ot_{c}_{pr}")
                for (j, pt, co, oo, w, st, sp) in pv_list:
                    nc.tensor.matmul(ots[:, oo:oo + w], v_nat[:, j, :],
                                     pt[:, co:co + w], start=st, stop=sp)
                nc.vector.tensor_mul(onorm[:, c, 2 * pr:2 * pr + 2, :], ots, rec)

            def emit_po(c, half):
                # output projection for two of chunk c's token tiles; each
                # 1024-wide half-row leaves on a rotating DMA queue as soon
                # as its cast lands
                for i in (4 * c + 2 * half, 4 * c + 2 * half + 1):
                    isl = slice(i * P, (i + 1) * P)
                    for n2 in range(2):
                        if c == NCH - 1 and (2 * i + n2) % 2 == 1:
                            ps = psc.tile([P, 1024], f32, tag="sc",
                                          name=f"po_{i}_{n2}")
                        else:
                            ps = pvd.tile([P, 1024], f32, tag="pvd",
                                          name=f"po_{i}_{n2}")
                        for q5 in range(2):
                            n5 = n2 * 1024 + q5 * 512
                            for h in range(GQ):
                                nc.tensor.matmul(
                                    ps[:, q5 * 512:(q5 + 1) * 512],
                                    onorm[:, i // 4, h, (i % 4) * P:(i % 4 + 1) * P],
                                    wo_sb[:, h, n5:n5 + 512],
                                    start=(h == 0), stop=(h == GQ - 1))
                        row = rowp.tile([P, 1024], bf16, tag="row",
                                        name=f"row_{i}_{n2}")
                        if (2 * i + n2) % 2 == 0:
                            nc.vector.tensor_copy(row, ps)
                        else:
                            nc.scalar.copy(row, ps)
                        eng = (nc.gpsimd, nc.scalar, nc.sync)[qrot[0] % 3]
                        qrot[0] += 1
                        eng.dma_start(out=po[isl, n2 * 1024:(n2 + 1) * 1024],
                                      in_=row)

            for c in range(NCH):
                if c == 0:
                    pass2(0, 0)
                    pass2(0, 1)
                else:
                    pass1(c, 0)
                    emit_po(c - 1, 0)
                    pass1(c, 1)
                    pass2(c, 0)
                    emit_po(c - 1, 1)
                    pass2(c, 1)
            emit_po(NCH - 1, 0)
            emit_po(NCH - 1, 1)
    nc.compile()
    return nc


def _causal_ok(mask):
    m = np.asarray(mask).reshape(S, S)
    tri = np.tril(np.ones((S, S), dtype=bool))
    return bool(np.all(m[tri] == 0.0) and np.all(m[~tri] <= -1e8))


def _reference_fallback(x, Wq, Wk, Wv, Wo, qg, kg, cos, sin, mask):
    x64 = np.asarray(x, dtype=np.float32)
    q = (x64 @ Wq).reshape(B, S, H, HD).transpose(0, 2, 1, 3)
    k = (x64 @ Wk).reshape(B, S, KV, HD).transpose(0, 2, 1, 3)
    v = (x64 @ Wv).reshape(B, S, KV, HD).transpose(0, 2, 1, 3)

    def rms(t, g):
        r = np.sqrt(np.mean(t * t, axis=-1, keepdims=True) + EPS)
        return g * (t / r)

    q, k = rms(q, qg), rms(k, kg)

    def rot(t):
        return np.concatenate([-t[..., HD // 2:], t[..., :HD // 2]], axis=-1)

    c = cos[None, None, :, :]
    s = sin[None, None, :, :]
    q = q * c + rot(q) * s
    k = k * c + rot(k) * s
    k = np.repeat(k, GQ, axis=1)
    v = np.repeat(v, GQ, axis=1)
    sc = np.einsum('bhqd,bhkd->bhqk', q, k) / np.sqrt(HD) + np.asarray(mask).reshape(1, 1, S, S)
    sc = sc - sc.max(axis=-1, keepdims=True)
    e = np.exp(sc)
    a = e / e.sum(axis=-1, keepdims=True)
    o = np.einsum('bhqk,bhkd->bhqd', a, v)
    o = o.transpose(0, 2, 1, 3).reshape(B, S, H * HD)
    return (o @ Wo).astype(np.float32)


def kernel(x, Wq, Wk, Wv, Wo, qg, kg, cos, sin, mask, **_unused):
    x = np.asarray(x, dtype=np.float32)
    Wq, Wk, Wv, Wo = (np.asarray(a, dtype=np.float32) for a in (Wq, Wk, Wv, Wo))
    qg, kg = np.asarray(qg, np.float32), np.asarray(kg, np.float32)
    cos, sin = np.asarray(cos, np.float32), np.asarray(sin, np.float32)
    if not _causal_ok(mask):
        return _reference_fallback(x, Wq, Wk, Wv, Wo, qg, kg, cos, sin, mask)

    from concourse.bass_utils import run_bass_kernel_spmd

    if "nc" not in _CACHED:
        _CACHED["nc"] = _build_program()
    nc = _CACHED["nc"]

    cosT = np.ascontiguousarray(cos.T)  # [HD, S]
    sinT = np.ascontiguousarray(sin.T)

    # rope via halves: out[:64] = x[:64]*cos[:64] + x[64:]*sin_tbl[:64]
    #                  out[64:] = x[64:]*cos[64:] + x[:64]*sin_tbl[64:]
    # reference: rot(x)[:64] = -x[64:], rot(x)[64:] = x[:64]; gains fold in.
    def tables(g):
        ct = cosT * g[:, None]
        st = np.empty_like(sinT)
        st[:64] = -sinT[:64] * g[64:, None]
        st[64:] = sinT[64:] * g[:64, None]
        return ct.astype(BF), st.astype(BF)

    cq, sq = tables(qg)
    ck, sk = tables(kg)

    rsw = np.zeros((P, P), dtype=np.float32)
    for i in range(P):
        rsw[i, (i + 64) % P] = 1.0
    rsw = rsw.astype(BF)

    # restricted-diagonal masks: within each 128-column sub-range that
    # starts a diagonal tile, query-col >= key-row; elsewhere 1.
    rows = np.arange(P)[:, None]
    tri = (np.arange(P)[None, :] >= rows)          # [128,128] step
    onesP = np.ones((P, P), dtype=bool)
    mA = np.concatenate([tri, onesP, onesP, onesP, tri, onesP, onesP], axis=1)  # 896
    mB = np.concatenate([tri, onesP, tri], axis=1)                              # 384
    mtri = np.concatenate([mA, mB], axis=1).astype(BF)                          # [128,1280]

    def part_layout(w, cols):
        # [DIM, cols] -> [P, NK, cols] with feature d = j*128 + p
        return np.ascontiguousarray(w.reshape(NK, P, cols).transpose(1, 0, 2)).astype(BF)

    xt4 = []
    for b in range(B):
        xT = x[b].T  # [DIM, S]
        xt4.append(np.ascontiguousarray(
            xT.reshape(NK, P, NCH, 512).transpose(1, 2, 0, 3)).astype(BF))

    in_maps = []
    for core in range(8):
        b, g = divmod(core, KV)
        wo_g = Wo[g * GQ * HD:(g + 1) * GQ * HD, :]
        wq_g = Wq[:, g * GQ * HD:(g + 1) * GQ * HD]  # [DIM, 4*HD]
        in_maps.append({
            "xt4": xt4[b],
            # head-major [P, GQ, NK, HD] so each head's slice is one
            # contiguous per-partition DMA
            "wq": np.ascontiguousarray(
                wq_g.reshape(NK, P, GQ, HD).transpose(1, 2, 0, 3)).astype(BF),
            "wk": part_layout(Wk[:, g * HD:(g + 1) * HD], HD),
            "wv": part_layout(Wv[:, g * HD:(g + 1) * HD], HD),
            "wo": np.ascontiguousarray(
                wo_g.reshape(GQ, P, DIM).transpose(1, 0, 2)).astype(BF),
            "cosq": cq, "sinq": sq, "cosk": ck, "sink": sk,
            "mtri": mtri, "rsw": rsw,
        })

    res = run_bass_kernel_spmd(nc, in_maps, list(range(8)))
    out = np.zeros((B, S, DIM), dtype=np.float32)
    for core in range(8):
        out[core // KV] += res.results[core]["po"].astype(np.float32)
    return out
